# revision 14
# baseline (speedup 1.0000x reference)
"""Trainium2 kernel for nn_MultiHeadCrossAttention_28063316313030.

Math: with seq_len == 1, softmax over a size-1 axis is identically 1, so
attention(Q,K,V) == V and W_Q/W_K are dead code.  The whole module collapses to

    out = LN(x1 @ A) + LN(x2 @ A),   A = W_V.T @ W_fc.T   (1024 x 1024)

where LN is LayerNorm over the last dim with gamma/beta.

Distribution: pure data parallel over the batch dim across 8 NeuronCores.
Host precomputes A (tiny matmul) and pre-tiles x1/x2 C-major so the TensorE
contraction dim lands on SBUF partitions with fully contiguous DMA runs.

Device per core (2048 rows per stream):
  warmup: ~28 dummy matmuls warm the PE clock (HAM) while DMAs fill.
  for each 128-row tile, for each stream:
    z = xT_tile.T @ A        (f32r matmuls, 8 k-tiles x 2 PSUM banks of 512)
    bn_stats/bn_aggr -> mean/var;  r = 1/sqrt(var+eps) (ACT sqrt + DVE recip)
    n = z*r - mu*r           (ACT Identity with per-partition scale/bias)
  out_tile = n1 + n2 (DVE), optional gamma/beta affine, DMA out.
"""

import sys

sys.path.insert(0, "/opt/trn_rl_repo")

import numpy as np

B, C, OUT = 16384, 1024, 1024
EPS = 1e-5
NCORES = 8
R = B // NCORES  # rows per core per stream
P = 128
KT = C // P  # contraction tiles
BT = R // P  # row tiles per core
NH = OUT // 512  # psum bank halves per row tile
N_WARMUP = 10

_cache = {}


def _build(use_affine: bool, mm_dtype_name: str):
    import concourse.bacc as bacc
    import concourse.bass as bass
    import concourse.mybir as mybir
    from concourse.tile import TileContext

    f32 = mybir.dt.float32
    mmdt = getattr(mybir.dt, mm_dtype_name)
    AF = mybir.ActivationFunctionType
    ALU = mybir.AluOpType

    nc = bacc.Bacc("TRN2", target_bir_lowering=False, debug=False, num_devices=NCORES)

    # host-pretiled: [ki, bt, ko, bi]
    x1p = nc.declare_dram_parameter("x1p", [P, BT, KT, P], mmdt, isOutput=False)
    x2p = nc.declare_dram_parameter("x2p", [P, BT, KT, P], mmdt, isOutput=False)
    # host-pretiled: [ki, ko, o]
    a_d = nc.declare_dram_parameter("a", [P, KT, OUT], mmdt, isOutput=False)
    if use_affine:
        gam_d = nc.declare_dram_parameter("gamma", [OUT], f32, isOutput=False)
        bet2_d = nc.declare_dram_parameter("beta2", [OUT], f32, isOutput=False)
    y_d = nc.declare_dram_parameter("y", [R, OUT], f32, isOutput=True)

    with TileContext(nc) as tc:
        with (
            tc.tile_pool(name="singles", bufs=1) as singles,
            tc.tile_pool(name="xs", bufs=3) as xpool,
            tc.tile_pool(name="ns", bufs=3) as npool,
            tc.tile_pool(name="outs", bufs=3) as opool,
            tc.tile_pool(name="stats", bufs=4) as stats,
            tc.tile_pool(name="psum", bufs=2, space="PSUM") as psum,
        ):
            # --- PE warmup: dummy matmuls with no input deps keep the PE busy
            # from t~0 so the HAM clock gate opens before real matmuls arrive.
            # --- first A chunk lands first on the sync ring; the PE warmup
            # matmuls read it directly (no memset dependency), run at ~100%
            # duty (N=512) so the HAM clock gate opens before real matmuls.
            a_sb = [[None] * NH for _ in range(KT)]
            a00 = singles.tile([P, 512], mmdt, tag="a0_0", name="a0_0")
            nc.sync.dma_start(a00[:], a_d[:, 0, 0:512])
            a_sb[0][0] = a00

            warm_ps = psum.tile([P, 512], f32, tag="ps00")
            for _ in range(N_WARMUP):
                nc.tensor.matmul(
                    warm_ps[:], lhsT=a00[:, :P], rhs=a00[:],
                    start=True, stop=True,
                )

            # --- bt=0 x tiles next on the sync ring.
            xt0 = []
            for s, xp in enumerate((x1p, x2p)):
                t = xpool.tile([P, KT, P], mmdt, tag=f"xt{s}", name=f"xt0_{s}")
                nc.sync.dma_start(t[:], xp[:, 0])
                xt0.append(t)

            # --- rest of A on the scalar HWDGE ring, in consumption order,
            # racing in parallel with the x loads on the sync ring.
            for k in range(KT):
                for h in range(NH):
                    if k == 0 and h == 0:
                        continue
                    t = singles.tile([P, 512], mmdt, tag=f"a{k}_{h}", name=f"a{k}_{h}")
                    nc.scalar.dma_start(t[:], a_d[:, k, h * 512 : (h + 1) * 512])
                    a_sb[k][h] = t

            eps_sb = singles.tile([P, 1], f32)
            nc.vector.memset(eps_sb, EPS)
            if use_affine:
                gam_sb = singles.tile([P, OUT], f32)
                nc.sync.dma_start(
                    gam_sb[:],
                    bass.AP(
                        tensor=gam_d.tensor,
                        offset=gam_d.offset,
                        ap=[[0, P], gam_d.ap[0]],
                    ),
                )
                bet2_sb = singles.tile([P, OUT], f32)
                nc.sync.dma_start(
                    bet2_sb[:],
                    bass.AP(
                        tensor=bet2_d.tensor,
                        offset=bet2_d.offset,
                        ap=[[0, P], bet2_d.ap[0]],
                    ),
                )

            for bt in range(BT):
                n_tiles = []
                for s, xp in enumerate((x1p, x2p)):
                    if bt == 0:
                        xt = xt0[s]
                    else:
                        xt = xpool.tile([P, KT, P], mmdt, tag=f"xt{s}")
                        nc.sync.dma_start(xt[:], xp[:, bt])

                    ps_tiles = [
                        psum.tile([P, 512], f32, tag=f"ps{s}{h}", name=f"ps{s}{h}")
                        for h in range(NH)
                    ]
                    for k in range(KT):
                        for h in range(NH):
                            nc.tensor.matmul(
                                ps_tiles[h][:],
                                lhsT=xt[:, k, :],
                                rhs=a_sb[k][h][:],
                                start=(k == 0),
                                stop=(k == KT - 1),
                            )

                    st = stats.tile([P, NH, 6], f32, tag=f"st{s}")
                    for h in range(NH):
                        nc.vector.bn_stats(st[:, h, :], ps_tiles[h][:])
                    mv = stats.tile([P, 2], f32, tag=f"mv{s}")
                    nc.vector.bn_aggr(mv[:], st[:])

                    # r = 1/sqrt(var + eps)
                    r_sb = stats.tile([P, 1], f32, tag=f"r{s}")
                    nc.scalar.activation(
                        r_sb[:], mv[:, 1:2], func=AF.Sqrt, bias=eps_sb[:], scale=1.0
                    )
                    nc.vector.reciprocal(r_sb[:], r_sb[:])
                    # nmr = -mean * r
                    nmr = stats.tile([P, 1], f32, tag=f"nmr{s}")
                    nc.vector.tensor_scalar(
                        nmr[:],
                        mv[:, 0:1],
                        scalar1=r_sb[:],
                        scalar2=-1.0,
                        op0=ALU.mult,
                        op1=ALU.mult,
                    )

                    ntile = npool.tile([P, OUT], f32, tag=f"n{s}")
                    for h in range(NH):
                        nc.scalar.activation(
                            ntile[:, h * 512 : (h + 1) * 512],
                            ps_tiles[h][:],
                            func=AF.Identity,
                            bias=nmr[:],
                            scale=r_sb[:],
                        )
                    n_tiles.append(ntile)

                # Per-half add + store so the kernel tail pipelines.
                out_t = opool.tile([P, OUT], f32, tag="out")
                for h in range(NH):
                    sl = slice(h * 512, (h + 1) * 512)
                    nc.vector.tensor_tensor(
                        out_t[:, sl], n_tiles[0][:, sl], n_tiles[1][:, sl],
                        op=ALU.add,
                    )
                    if use_affine:
                        nc.vector.tensor_tensor(
                            out_t[:, sl], out_t[:, sl], gam_sb[:, sl], op=ALU.mult
                        )
                        nc.vector.tensor_tensor(
                            out_t[:, sl], out_t[:, sl], bet2_sb[:, sl], op=ALU.add
                        )
                    nc.sync.dma_start(
                        y_d[bt * P : (bt + 1) * P, sl], out_t[:, sl]
                    )

    nc.finalize()
    return nc


def _get_nc(use_affine: bool, mm_dtype_name: str):
    key = (use_affine, mm_dtype_name)
    if key not in _cache:
        _cache[key] = _build(use_affine, mm_dtype_name)
    return _cache[key]


def _pretile_x(x_core: np.ndarray) -> np.ndarray:
    # [R, C] -> [ki, bt, ko, bi]
    return np.ascontiguousarray(
        x_core.reshape(BT, P, KT, P).transpose(3, 0, 2, 1)
    )


def kernel(x1, x2, W_Q, W_K, W_V, W_fc, gamma, beta, _trace=False,
           _mm_dtype="float32r"):
    from concourse.bass_utils import run_bass_kernel_spmd

    x1 = np.asarray(x1, dtype=np.float32)
    x2 = np.asarray(x2, dtype=np.float32)
    W_V = np.asarray(W_V, dtype=np.float32)
    W_fc = np.asarray(W_fc, dtype=np.float32)
    gamma = np.asarray(gamma, dtype=np.float32)
    beta = np.asarray(beta, dtype=np.float32)

    # A = W_V.T @ W_fc.T in float64 to keep the host collapse error negligible.
    A = (W_V.T.astype(np.float64) @ W_fc.T.astype(np.float64)).astype(np.float32)
    # [C, OUT] -> [ki, ko, o]
    Ap = np.ascontiguousarray(A.reshape(KT, P, OUT).transpose(1, 0, 2))

    use_affine = not (np.all(gamma == 1.0) and np.all(beta == 0.0))

    in_maps = []
    for r in range(NCORES):
        sl = slice(r * R, (r + 1) * R)
        m = {
            "x1p": _pretile_x(x1[sl]),
            "x2p": _pretile_x(x2[sl]),
            "a": Ap,
        }
        if use_affine:
            m["gamma"] = gamma
            m["beta2"] = (2.0 * beta).astype(np.float32)
        in_maps.append(m)

    nc = _get_nc(use_affine, _mm_dtype)
    res = run_bass_kernel_spmd(nc, in_maps, list(range(NCORES)), trace=_trace)

    y = np.concatenate([res.results[r]["y"] for r in range(NCORES)], axis=0)
    out = y.reshape(B, 1, OUT)
    if _trace:
        return out, res
    return out


# revision 15
# speedup vs baseline: 1.0276x; 1.0276x over previous
"""Trainium2 kernel for nn_MultiHeadCrossAttention_28063316313030.

Math: with seq_len == 1, softmax over a size-1 axis is identically 1, so
attention(Q,K,V) == V and W_Q/W_K are dead code.  The whole module collapses to

    out = LN(x1 @ A) + LN(x2 @ A),   A = W_V.T @ W_fc.T   (1024 x 1024)

where LN is LayerNorm over the last dim with gamma/beta.

Distribution: pure data parallel over the batch dim across 8 NeuronCores.
Host precomputes A (tiny matmul) and pre-tiles x1/x2 C-major so the TensorE
contraction dim lands on SBUF partitions with fully contiguous DMA runs.

Device per core (2048 rows per stream):
  warmup: ~28 dummy matmuls warm the PE clock (HAM) while DMAs fill.
  for each 128-row tile, for each stream:
    z = xT_tile.T @ A        (f32r matmuls, 8 k-tiles x 2 PSUM banks of 512)
    bn_stats/bn_aggr -> mean/var;  r = 1/sqrt(var+eps) (ACT sqrt + DVE recip)
    n = z*r - mu*r           (ACT Identity with per-partition scale/bias)
  out_tile = n1 + n2 (DVE), optional gamma/beta affine, DMA out.
"""

import sys

sys.path.insert(0, "/opt/trn_rl_repo")

import numpy as np

B, C, OUT = 16384, 1024, 1024
EPS = 1e-5
NCORES = 8
R = B // NCORES  # rows per core per stream
P = 128
KT = C // P  # contraction tiles
BT = R // P  # row tiles per core
NH = OUT // 512  # psum bank halves per row tile
N_WARMUP = 10

_cache = {}


def _build(use_affine: bool, mm_dtype_name: str):
    import concourse.bacc as bacc
    import concourse.bass as bass
    import concourse.mybir as mybir
    from concourse.tile import TileContext

    f32 = mybir.dt.float32
    mmdt = getattr(mybir.dt, mm_dtype_name)
    AF = mybir.ActivationFunctionType
    ALU = mybir.AluOpType

    nc = bacc.Bacc("TRN2", target_bir_lowering=False, debug=False, num_devices=NCORES)

    # host-pretiled: [ki, bt, ko, bi]
    x1p = nc.declare_dram_parameter("x1p", [P, BT, KT, P], mmdt, isOutput=False)
    x2p = nc.declare_dram_parameter("x2p", [P, BT, KT, P], mmdt, isOutput=False)
    # host-pretiled: [ki, ko, o]
    a_d = nc.declare_dram_parameter("a", [P, KT, OUT], mmdt, isOutput=False)
    if use_affine:
        gam_d = nc.declare_dram_parameter("gamma", [OUT], f32, isOutput=False)
        bet2_d = nc.declare_dram_parameter("beta2", [OUT], f32, isOutput=False)
    y_d = nc.declare_dram_parameter("y", [R, OUT], f32, isOutput=True)

    with TileContext(nc) as tc:
        with (
            tc.tile_pool(name="singles", bufs=1) as singles,
            tc.tile_pool(name="xs", bufs=3) as xpool,
            tc.tile_pool(name="ns", bufs=3) as npool,
            tc.tile_pool(name="outs", bufs=3) as opool,
            tc.tile_pool(name="stats", bufs=4) as stats,
            tc.tile_pool(name="psum", bufs=2, space="PSUM") as psum,
        ):
            # --- PE warmup: dummy matmuls with no input deps keep the PE busy
            # from t~0 so the HAM clock gate opens before real matmuls arrive.
            # --- first A chunk lands first on the sync ring; the PE warmup
            # matmuls read it directly (no memset dependency), run at ~100%
            # duty (N=512) so the HAM clock gate opens before real matmuls.
            a_sb = [[None] * NH for _ in range(KT)]
            a00 = singles.tile([P, 512], mmdt, tag="a0_0", name="a0_0")
            nc.sync.dma_start(a00[:], a_d[:, 0, 0:512])
            a_sb[0][0] = a00

            warm_ps = psum.tile([P, 512], f32, tag="ps00")
            for _ in range(N_WARMUP):
                nc.tensor.matmul(
                    warm_ps[:], lhsT=a00[:, :P], rhs=a00[:],
                    start=True, stop=True,
                )

            # --- bt=0 x tiles next on the sync ring.
            xt0 = []
            for s, xp in enumerate((x1p, x2p)):
                t = xpool.tile([P, KT, P], mmdt, tag=f"xt{s}", name=f"xt0_{s}")
                nc.sync.dma_start(t[:], xp[:, 0])
                xt0.append(t)

            # --- rest of A on the same ring, in consumption order.
            for k in range(KT):
                for h in range(NH):
                    if k == 0 and h == 0:
                        continue
                    t = singles.tile([P, 512], mmdt, tag=f"a{k}_{h}", name=f"a{k}_{h}")
                    nc.sync.dma_start(t[:], a_d[:, k, h * 512 : (h + 1) * 512])
                    a_sb[k][h] = t

            eps_sb = singles.tile([P, 1], f32)
            nc.vector.memset(eps_sb, EPS)
            if use_affine:
                gam_sb = singles.tile([P, OUT], f32)
                nc.sync.dma_start(
                    gam_sb[:],
                    bass.AP(
                        tensor=gam_d.tensor,
                        offset=gam_d.offset,
                        ap=[[0, P], gam_d.ap[0]],
                    ),
                )
                bet2_sb = singles.tile([P, OUT], f32)
                nc.sync.dma_start(
                    bet2_sb[:],
                    bass.AP(
                        tensor=bet2_d.tensor,
                        offset=bet2_d.offset,
                        ap=[[0, P], bet2_d.ap[0]],
                    ),
                )

            for bt in range(BT):
                n_tiles = []
                for s, xp in enumerate((x1p, x2p)):
                    if bt == 0:
                        xt = xt0[s]
                    else:
                        xt = xpool.tile([P, KT, P], mmdt, tag=f"xt{s}")
                        nc.sync.dma_start(xt[:], xp[:, bt])

                    ps_tiles = [
                        psum.tile([P, 512], f32, tag=f"ps{s}{h}", name=f"ps{s}{h}")
                        for h in range(NH)
                    ]
                    for k in range(KT):
                        for h in range(NH):
                            nc.tensor.matmul(
                                ps_tiles[h][:],
                                lhsT=xt[:, k, :],
                                rhs=a_sb[k][h][:],
                                start=(k == 0),
                                stop=(k == KT - 1),
                            )

                    st = stats.tile([P, NH, 6], f32, tag=f"st{s}")
                    for h in range(NH):
                        nc.vector.bn_stats(st[:, h, :], ps_tiles[h][:])
                    mv = stats.tile([P, 2], f32, tag=f"mv{s}")
                    nc.vector.bn_aggr(mv[:], st[:])

                    # r = 1/sqrt(var + eps)
                    r_sb = stats.tile([P, 1], f32, tag=f"r{s}")
                    nc.scalar.activation(
                        r_sb[:], mv[:, 1:2], func=AF.Sqrt, bias=eps_sb[:], scale=1.0
                    )
                    nc.vector.reciprocal(r_sb[:], r_sb[:])
                    # nmr = -mean * r
                    nmr = stats.tile([P, 1], f32, tag=f"nmr{s}")
                    nc.vector.tensor_scalar(
                        nmr[:],
                        mv[:, 0:1],
                        scalar1=r_sb[:],
                        scalar2=-1.0,
                        op0=ALU.mult,
                        op1=ALU.mult,
                    )

                    ntile = npool.tile([P, OUT], f32, tag=f"n{s}")
                    for h in range(NH):
                        nc.scalar.activation(
                            ntile[:, h * 512 : (h + 1) * 512],
                            ps_tiles[h][:],
                            func=AF.Identity,
                            bias=nmr[:],
                            scale=r_sb[:],
                        )
                    n_tiles.append(ntile)

                # Per-half add + store so the kernel tail pipelines.
                out_t = opool.tile([P, OUT], f32, tag="out")
                for h in range(NH):
                    sl = slice(h * 512, (h + 1) * 512)
                    nc.vector.tensor_tensor(
                        out_t[:, sl], n_tiles[0][:, sl], n_tiles[1][:, sl],
                        op=ALU.add,
                    )
                    if use_affine:
                        nc.vector.tensor_tensor(
                            out_t[:, sl], out_t[:, sl], gam_sb[:, sl], op=ALU.mult
                        )
                        nc.vector.tensor_tensor(
                            out_t[:, sl], out_t[:, sl], bet2_sb[:, sl], op=ALU.add
                        )
                    nc.sync.dma_start(
                        y_d[bt * P : (bt + 1) * P, sl], out_t[:, sl]
                    )

    nc.finalize()
    return nc


def _get_nc(use_affine: bool, mm_dtype_name: str):
    key = (use_affine, mm_dtype_name)
    if key not in _cache:
        _cache[key] = _build(use_affine, mm_dtype_name)
    return _cache[key]


def _pretile_x(x_core: np.ndarray) -> np.ndarray:
    # [R, C] -> [ki, bt, ko, bi]
    return np.ascontiguousarray(
        x_core.reshape(BT, P, KT, P).transpose(3, 0, 2, 1)
    )


def kernel(x1, x2, W_Q, W_K, W_V, W_fc, gamma, beta, _trace=False,
           _mm_dtype="float32r"):
    from concourse.bass_utils import run_bass_kernel_spmd

    x1 = np.asarray(x1, dtype=np.float32)
    x2 = np.asarray(x2, dtype=np.float32)
    W_V = np.asarray(W_V, dtype=np.float32)
    W_fc = np.asarray(W_fc, dtype=np.float32)
    gamma = np.asarray(gamma, dtype=np.float32)
    beta = np.asarray(beta, dtype=np.float32)

    # A = W_V.T @ W_fc.T in float64 to keep the host collapse error negligible.
    A = (W_V.T.astype(np.float64) @ W_fc.T.astype(np.float64)).astype(np.float32)
    # [C, OUT] -> [ki, ko, o]
    Ap = np.ascontiguousarray(A.reshape(KT, P, OUT).transpose(1, 0, 2))

    use_affine = not (np.all(gamma == 1.0) and np.all(beta == 0.0))

    in_maps = []
    for r in range(NCORES):
        sl = slice(r * R, (r + 1) * R)
        m = {
            "x1p": _pretile_x(x1[sl]),
            "x2p": _pretile_x(x2[sl]),
            "a": Ap,
        }
        if use_affine:
            m["gamma"] = gamma
            m["beta2"] = (2.0 * beta).astype(np.float32)
        in_maps.append(m)

    nc = _get_nc(use_affine, _mm_dtype)
    res = run_bass_kernel_spmd(nc, in_maps, list(range(NCORES)), trace=_trace)

    y = np.concatenate([res.results[r]["y"] for r in range(NCORES)], axis=0)
    out = y.reshape(B, 1, OUT)
    if _trace:
        return out, res
    return out


# revision 18
# speedup vs baseline: 1.0366x; 1.0087x over previous
"""Trainium2 kernel for nn_MultiHeadCrossAttention_28063316313030.

Math: with seq_len == 1, softmax over a size-1 axis is identically 1, so
attention(Q,K,V) == V and W_Q/W_K are dead code.  The whole module collapses to

    out = LN(x1 @ A) + LN(x2 @ A),   A = W_V.T @ W_fc.T   (1024 x 1024)

where LN is LayerNorm over the last dim with gamma/beta.

Distribution: pure data parallel over the batch dim across 8 NeuronCores.
Host precomputes A (tiny matmul) and pre-tiles x1/x2 C-major so the TensorE
contraction dim lands on SBUF partitions with fully contiguous DMA runs.

Device per core (2048 rows per stream):
  warmup: ~28 dummy matmuls warm the PE clock (HAM) while DMAs fill.
  for each 128-row tile, for each stream:
    z = xT_tile.T @ A        (f32r matmuls, 8 k-tiles x 2 PSUM banks of 512)
    bn_stats/bn_aggr -> mean/var;  r = 1/sqrt(var+eps) (ACT sqrt + DVE recip)
    n = z*r - mu*r           (ACT Identity with per-partition scale/bias)
  out_tile = n1 + n2 (DVE), optional gamma/beta affine, DMA out.
"""

import sys

sys.path.insert(0, "/opt/trn_rl_repo")

import numpy as np

B, C, OUT = 16384, 1024, 1024
EPS = 1e-5
NCORES = 8
R = B // NCORES  # rows per core per stream
P = 128
KT = C // P  # contraction tiles
BT = R // P  # row tiles per core
NH = OUT // 512  # psum bank halves per row tile
N_WARMUP = 12

_cache = {}


def _build(use_affine: bool, mm_dtype_name: str):
    import concourse.bacc as bacc
    import concourse.bass as bass
    import concourse.mybir as mybir
    from concourse.tile import TileContext

    f32 = mybir.dt.float32
    mmdt = getattr(mybir.dt, mm_dtype_name)
    AF = mybir.ActivationFunctionType
    ALU = mybir.AluOpType

    nc = bacc.Bacc("TRN2", target_bir_lowering=False, debug=False, num_devices=NCORES)

    # host-pretiled: [ki, bt, ko, bi]
    x1p = nc.declare_dram_parameter("x1p", [P, BT, KT, P], mmdt, isOutput=False)
    x2p = nc.declare_dram_parameter("x2p", [P, BT, KT, P], mmdt, isOutput=False)
    # host-pretiled: [ki, ko, o]
    a_d = nc.declare_dram_parameter("a", [P, KT, OUT], mmdt, isOutput=False)
    if use_affine:
        gam_d = nc.declare_dram_parameter("gamma", [OUT], f32, isOutput=False)
        bet2_d = nc.declare_dram_parameter("beta2", [OUT], f32, isOutput=False)
    y_d = nc.declare_dram_parameter("y", [R, OUT], f32, isOutput=True)

    with TileContext(nc) as tc:
        with (
            tc.tile_pool(name="singles", bufs=1) as singles,
            tc.tile_pool(name="xs", bufs=3) as xpool,
            tc.tile_pool(name="ns", bufs=3) as npool,
            tc.tile_pool(name="outs", bufs=3) as opool,
            tc.tile_pool(name="stats", bufs=4) as stats,
            tc.tile_pool(name="psum", bufs=2, space="PSUM") as psum,
        ):
            # --- PE warmup: dummy matmuls with no input deps keep the PE busy
            # from t~0 so the HAM clock gate opens before real matmuls arrive.
            # --- tiny warm tile lands first on the ring; PE warmup matmuls
            # (full-duty N=256) trip the HAM activity window so the clock
            # gate opens before the real matmuls arrive.
            warm_sb = singles.tile([P, 256], mmdt)
            nc.sync.dma_start(warm_sb[:], a_d[:, 0, 0:256])
            warm_ps = psum.tile([P, 512], f32, tag="ps00")
            for _ in range(N_WARMUP):
                nc.tensor.matmul(
                    warm_ps[:, :256], lhsT=warm_sb[:, :P], rhs=warm_sb[:],
                    start=True, stop=True,
                )

            # --- first A chunk next: the first real matmul needs only this
            # plus the first x tile, not the whole 4MB of A.
            a_sb = [[None] * NH for _ in range(KT)]
            a00 = singles.tile([P, 512], mmdt, tag="a0_0", name="a0_0")
            nc.sync.dma_start(a00[:], a_d[:, 0, 0:512])
            a_sb[0][0] = a00

            # --- bt=0 x tiles next on the sync ring.
            xt0 = []
            for s, xp in enumerate((x1p, x2p)):
                t = xpool.tile([P, KT, P], mmdt, tag=f"xt{s}", name=f"xt0_{s}")
                nc.sync.dma_start(t[:], xp[:, 0])
                xt0.append(t)

            # --- rest of A on the same ring, in consumption order.
            for k in range(KT):
                for h in range(NH):
                    if k == 0 and h == 0:
                        continue
                    t = singles.tile([P, 512], mmdt, tag=f"a{k}_{h}", name=f"a{k}_{h}")
                    nc.sync.dma_start(t[:], a_d[:, k, h * 512 : (h + 1) * 512])
                    a_sb[k][h] = t

            eps_sb = singles.tile([P, 1], f32)
            nc.vector.memset(eps_sb, EPS)
            if use_affine:
                gam_sb = singles.tile([P, OUT], f32)
                nc.sync.dma_start(
                    gam_sb[:],
                    bass.AP(
                        tensor=gam_d.tensor,
                        offset=gam_d.offset,
                        ap=[[0, P], gam_d.ap[0]],
                    ),
                )
                bet2_sb = singles.tile([P, OUT], f32)
                nc.sync.dma_start(
                    bet2_sb[:],
                    bass.AP(
                        tensor=bet2_d.tensor,
                        offset=bet2_d.offset,
                        ap=[[0, P], bet2_d.ap[0]],
                    ),
                )

            for bt in range(BT):
                n_tiles = []
                for s, xp in enumerate((x1p, x2p)):
                    if bt == 0:
                        xt = xt0[s]
                    else:
                        xt = xpool.tile([P, KT, P], mmdt, tag=f"xt{s}")
                        nc.sync.dma_start(xt[:], xp[:, bt])

                    ps_tiles = [
                        psum.tile([P, 512], f32, tag=f"ps{s}{h}", name=f"ps{s}{h}")
                        for h in range(NH)
                    ]
                    last = bt == BT - 1 and s == 1
                    if last:
                        # h-outer: h=0 finishes 8 matmuls early, so its
                        # bn_stats overlaps h=1's matmuls -> shorter tail.
                        for h in range(NH):
                            for k in range(KT):
                                nc.tensor.matmul(
                                    ps_tiles[h][:],
                                    lhsT=xt[:, k, :],
                                    rhs=a_sb[k][h][:],
                                    start=(k == 0),
                                    stop=(k == KT - 1),
                                )
                    else:
                        for k in range(KT):
                            for h in range(NH):
                                nc.tensor.matmul(
                                    ps_tiles[h][:],
                                    lhsT=xt[:, k, :],
                                    rhs=a_sb[k][h][:],
                                    start=(k == 0),
                                    stop=(k == KT - 1),
                                )

                    st = stats.tile([P, NH, 6], f32, tag=f"st{s}")
                    for h in range(NH):
                        nc.vector.bn_stats(st[:, h, :], ps_tiles[h][:])
                    mv = stats.tile([P, 2], f32, tag=f"mv{s}")
                    nc.vector.bn_aggr(mv[:], st[:])

                    # r = 1/sqrt(var + eps)
                    r_sb = stats.tile([P, 1], f32, tag=f"r{s}")
                    nc.scalar.activation(
                        r_sb[:], mv[:, 1:2], func=AF.Sqrt, bias=eps_sb[:], scale=1.0
                    )
                    nc.vector.reciprocal(r_sb[:], r_sb[:])
                    # nmr = -mean * r
                    nmr = stats.tile([P, 1], f32, tag=f"nmr{s}")
                    nc.vector.tensor_scalar(
                        nmr[:],
                        mv[:, 0:1],
                        scalar1=r_sb[:],
                        scalar2=-1.0,
                        op0=ALU.mult,
                        op1=ALU.mult,
                    )

                    ntile = npool.tile([P, OUT], f32, tag=f"n{s}")
                    for h in range(NH):
                        nc.scalar.activation(
                            ntile[:, h * 512 : (h + 1) * 512],
                            ps_tiles[h][:],
                            func=AF.Identity,
                            bias=nmr[:],
                            scale=r_sb[:],
                        )
                    n_tiles.append(ntile)

                # Per-half add + store so the kernel tail pipelines.
                out_t = opool.tile([P, OUT], f32, tag="out")
                for h in range(NH):
                    sl = slice(h * 512, (h + 1) * 512)
                    nc.vector.tensor_tensor(
                        out_t[:, sl], n_tiles[0][:, sl], n_tiles[1][:, sl],
                        op=ALU.add,
                    )
                    if use_affine:
                        nc.vector.tensor_tensor(
                            out_t[:, sl], out_t[:, sl], gam_sb[:, sl], op=ALU.mult
                        )
                        nc.vector.tensor_tensor(
                            out_t[:, sl], out_t[:, sl], bet2_sb[:, sl], op=ALU.add
                        )
                    nc.sync.dma_start(
                        y_d[bt * P : (bt + 1) * P, sl], out_t[:, sl]
                    )

    nc.finalize()
    return nc


def _get_nc(use_affine: bool, mm_dtype_name: str):
    key = (use_affine, mm_dtype_name)
    if key not in _cache:
        _cache[key] = _build(use_affine, mm_dtype_name)
    return _cache[key]


def _pretile_x(x_core: np.ndarray) -> np.ndarray:
    # [R, C] -> [ki, bt, ko, bi]
    return np.ascontiguousarray(
        x_core.reshape(BT, P, KT, P).transpose(3, 0, 2, 1)
    )


def kernel(x1, x2, W_Q, W_K, W_V, W_fc, gamma, beta, _trace=False,
           _mm_dtype="float32r"):
    from concourse.bass_utils import run_bass_kernel_spmd

    x1 = np.asarray(x1, dtype=np.float32)
    x2 = np.asarray(x2, dtype=np.float32)
    W_V = np.asarray(W_V, dtype=np.float32)
    W_fc = np.asarray(W_fc, dtype=np.float32)
    gamma = np.asarray(gamma, dtype=np.float32)
    beta = np.asarray(beta, dtype=np.float32)

    # A = W_V.T @ W_fc.T in float64 to keep the host collapse error negligible.
    A = (W_V.T.astype(np.float64) @ W_fc.T.astype(np.float64)).astype(np.float32)
    # [C, OUT] -> [ki, ko, o]
    Ap = np.ascontiguousarray(A.reshape(KT, P, OUT).transpose(1, 0, 2))

    use_affine = not (np.all(gamma == 1.0) and np.all(beta == 0.0))

    in_maps = []
    for r in range(NCORES):
        sl = slice(r * R, (r + 1) * R)
        m = {
            "x1p": _pretile_x(x1[sl]),
            "x2p": _pretile_x(x2[sl]),
            "a": Ap,
        }
        if use_affine:
            m["gamma"] = gamma
            m["beta2"] = (2.0 * beta).astype(np.float32)
        in_maps.append(m)

    nc = _get_nc(use_affine, _mm_dtype)
    res = run_bass_kernel_spmd(nc, in_maps, list(range(NCORES)), trace=_trace)

    y = np.concatenate([res.results[r]["y"] for r in range(NCORES)], axis=0)
    out = y.reshape(B, 1, OUT)
    if _trace:
        return out, res
    return out
